# revision 23
# baseline (speedup 1.0000x reference)
"""Trainium2 Bass kernel for an 8-layer GPT forward pass + CE loss.

Distribution (8 NeuronCores): token-sharded. Core c handles sequence b=c//4
(of B=2) and, within that sequence's 1024 tokens, two 128-token half-chunks
hc_r and hc_{7-r} (r=c%4) — the symmetric pairing balances causal-attention
cost across cores. Per layer, each core computes LN/QKV/FFN for its own 256
tokens only; K and V chunks are exchanged within each 4-core group via one
AllGather per layer. The LM head and the per-token CE terms are computed
token-sharded as well (no vocab sharding, so the softmax over V=32000 is
core-local). The host only shards inputs, concatenates the logit chunks and
averages the per-token loss terms.

Matmul operands are cast to bf16 (fp32 accumulation in PSUM); LN, softmax,
residual stream and logits stay fp32.
"""

import os
import sys
from contextlib import ExitStack

import numpy as np
import ml_dtypes

for _p in ("/opt/trn_rl_repo", "/root/.axon_site/_ro/trn_rl_repo"):
    if os.path.isdir(_p) and _p not in sys.path:
        sys.path.insert(0, _p)

import concourse.bass as bass
import concourse.mybir as mybir
import concourse.tile as tile
from concourse import bacc
import concourse.bass_utils as bass_utils
from concourse.masks import make_identity

F32 = mybir.dt.float32
BF16 = mybir.dt.bfloat16
I16 = mybir.dt.int16
AX = mybir.AxisListType
ALU = mybir.AluOpType
ACTF = mybir.ActivationFunctionType

BF = ml_dtypes.bfloat16

N_CORES = 8
EPS = 1e-5

# sbuf tile tag -> bufs (must be consistent per tag)
TB = {
    "xn_fm": 2, "st6": 2, "agg": 2, "sd": 2, "rinv": 2, "xn": 2, "xnb": 2,
    "wproj": 14, "q_fm": 1, "k_fm": 1, "v_tm": 1, "o_fm": 1,
    "k_all": 1, "v_all": 1,
    "probsT0": 3, "probsT1": 3, "probsa0": 3, "probsa1": 3, "srow": 3, "stot": 3, "sinv": 3,
    "pb0": 3, "pb1": 3,
    "w1": 10, "relu": 1, "w2": 3,
    "g1bc": 2, "b1bc": 2, "g2bc": 2, "b2bc": 2, "bobc": 2, "b2rbc": 2,
    "gfbc": 1, "bfbc": 1,
    "lm": 14, "lsb": 6, "esc": 4, "expacc": 1, "ones": 1, "lmb_sb": 1,
    "fstot": 2, "flse": 2, "grow": 2, "gidx": 2, "tm_sb": 2, "tl": 2,
    "prod": 2, "lt": 1,
}
PB = {"pbig": 4, "pmed": 2, "ptr": 2}


class Cfg:
    def __init__(self, D=1024, H=16, L=8, FF=4096, V=32000, VC=500, VG=8):
        self.D, self.H, self.L, self.FF, self.V = D, H, L, FF, V
        self.HD = 64
        assert D == self.HD * H
        self.DT = D // 128          # D tiles
        self.FT = FF // 128         # FF tiles
        self.VC = VC                # vocab chunk (psum free dim)
        self.VG = VG                # chunks per vocab group
        self.NVG = V // (VC * VG)   # vocab groups
        assert V == VC * VG * self.NVG
        assert (128 * V) % 512 == 0
        self.T = 1024               # tokens per sequence (8 half-chunks of 128)
        self.B = 2
        self.AGK = 256 * D          # K_fm part of allgather payload (elems)
        self.AGN = 2 * 256 * D      # total allgather payload per core (elems)
        # flags set from input values at build time
        self.use_ln_affine = False
        self.use_bo = False
        self.use_b1 = False
        self.use_b2 = False
        self.use_lnf_affine = False
        self.use_lm_b = False


def build(cfg: Cfg):
    nc = bacc.Bacc("TRN2", target_bir_lowering=False, debug=False,
                   num_devices=N_CORES)
    D, H, L, FF, V = cfg.D, cfg.H, cfg.L, cfg.FF, cfg.V
    DT, FT, VC, VG, NVG = cfg.DT, cfg.FT, cfg.VC, cfg.VG, cfg.NVG
    HD = cfg.HD
    NHP = H // 2  # head pairs
    MG = FT // 4  # W1 m-groups (4 m-tiles each)
    CW = min(512, D)  # output chunk width

    dram = lambda name, shape, dt, kind="ExternalInput": nc.dram_tensor(
        name, shape, dt, kind=kind).ap()

    tok_emb = dram("tok_emb", [V, D], F32)
    pos_sl = dram("pos_sl", [128, 2, D], F32)
    tok_idx = dram("tok_idx", [128, 16], I16)
    wq_d = dram("wq", [L, DT, 128, D], BF16)
    wk_d = dram("wk", [L, DT, 128, D], BF16)
    wv_d = dram("wv", [L, DT, 128, D], BF16)
    wo_d = dram("wo", [L, DT, 128, D], BF16)
    w1_d = dram("w1", [L, DT, MG, 128, 512], BF16)   # m-grouped tiles
    w2_d = dram("w2", [L, FT, 128, D], BF16)
    lm_d = dram("lmw", [DT, 128, V], BF16)
    mask0 = dram("mask0", [128, 4, 128], BF16)
    mask1 = dram("mask1", [128, 8, 128], BF16)
    tmask = dram("tmask", [128, 2, 512], F32)
    tgt_idx = dram("tgt_idx", [128, 16], I16)
    if cfg.use_ln_affine:
        lnp_d = dram("lnp", [L, 4, D], F32)  # ln1_g, ln1_b, ln2_g, ln2_b
    if cfg.use_lnf_affine:
        lnf_d = dram("lnf", [2, D], F32)
    if cfg.use_b1:
        b1f_d = dram("b1f", [128, L, FT], F32)
    if cfg.use_bo:
        bo_d = dram("bo_in", [L, D], F32)
    if cfg.use_b2:
        b2_d = dram("b2_in", [L, D], F32)
    if cfg.use_lm_b:
        lmb_d = dram("lmb", [V], BF16)

    logits_out = dram("logits_out", [256, V], F32, kind="ExternalOutput")
    if os.environ.get("KDEBUG"):
        hdbg = dram("h_dbg", [L + 1, 128, 2, D], F32, kind="ExternalOutput")
        hdbg2 = dram("h_dbg2", [L, 128, 2, D], F32, kind="ExternalOutput")
        odbg = dram("o_dbg", [L, 128, DT, 256], BF16, kind="ExternalOutput")
        xdbg = dram("x_dbg", [L, 128, DT, 256], BF16, kind="ExternalOutput")
        rdbg = dram("r_dbg", [L, 128, FT, 256], BF16, kind="ExternalOutput")
    loss_out = dram("loss_terms", [128, 2], F32, kind="ExternalOutput")

    with tile.TileContext(nc) as tc, ExitStack() as ctx:
        pers = ctx.enter_context(tc.tile_pool(name="pers", bufs=1))

        ident = pers.tile([128, 128], BF16, name="ident")
        make_identity(nc, ident[:])
        m0 = pers.tile([128, 4, 128], BF16, name="m0")
        nc.sync.dma_start(m0[:], mask0[:])
        m1 = pers.tile([128, 8, 128], BF16, name="m1")
        nc.sync.dma_start(m1[:], mask1[:])
        h = pers.tile([128, 2, D], F32, name="h")
        eps_t = pers.tile([128, 1], F32, name="eps_t")
        nc.gpsimd.memset(eps_t[:], EPS)
        if cfg.use_b1:
            b1f_sb = pers.tile([128, L, FT], F32, name="b1f_sb")
            nc.sync.dma_start(b1f_sb[:], b1f_d[:])

        # --- embedding: h = tok_emb[x] + pos ---
        idx_sb = pers.tile([128, 16], I16, name="idx_sb")
        nc.sync.dma_start(idx_sb[:], tok_idx[:])
        nc.gpsimd.dma_gather(h[:], tok_emb[:], idx_sb[:],
                             num_idxs=256, num_idxs_reg=256, elem_size=D)
        pos_sb = pers.tile([128, 2, D], F32, name="pos_sb")
        nc.sync.dma_start(pos_sb[:], pos_sl[:])
        nc.vector.tensor_add(h[:], h[:], pos_sb[:])
        if os.environ.get("KDEBUG"):
            nc.sync.dma_start(hdbg[0], h[:])

        bnb = min(512, D)  # bn_stats free-dim limit

        def tl_(pool, shape, dt, tag, name=None):
            return pool.tile(shape, dt, tag=tag, bufs=TB[tag],
                             name=name or tag)

        def pt_(pool, shape, dt, tag):
            return pool.tile(shape, dt, tag=tag, bufs=PB[tag], name=tag)

        def layer_norm(pool, pspool, g_bc, b_bc, name):
            """LN over h (token-major) -> xn_fm bf16 [128, DT, 256]."""
            xn_fm = tl_(pool, [128, DT, 256], BF16, "xn_fm", f"xnfm_{name}")
            for qt in range(2):
                hqt = h[:, qt, :]
                st6 = tl_(pool, [128, D // bnb, 6], F32, "st6")
                for a in range(D // bnb):
                    nc.vector.bn_stats(st6[:, a, :],
                                       hqt[:, a * bnb:(a + 1) * bnb])
                agg = tl_(pool, [128, 2], F32, "agg")
                nc.vector.bn_aggr(agg[:], st6[:])
                sd = tl_(pool, [128, 1], F32, "sd")
                nc.scalar.activation(sd[:], agg[:, 1:2], ACTF.Sqrt, bias=eps_t[:])
                rinv = tl_(pool, [128, 1], F32, "rinv")
                nc.vector.reciprocal(rinv[:], sd[:])
                xnb = tl_(pool, [128, D], BF16, "xnb")
                if g_bc is None and b_bc is None:
                    # fused: (h - mean) * rinv straight to bf16
                    nc.vector.tensor_scalar(xnb[:], hqt, agg[:, 0:1], rinv[:],
                                            op0=ALU.subtract, op1=ALU.mult)
                else:
                    xn = tl_(pool, [128, D], F32, "xn")
                    nc.vector.tensor_scalar(xn[:], hqt, agg[:, 0:1], rinv[:],
                                            op0=ALU.subtract, op1=ALU.mult)
                    if g_bc is not None:
                        nc.vector.tensor_mul(xn[:], xn[:], g_bc[:])
                    if b_bc is not None:
                        nc.vector.tensor_add(xn[:], xn[:], b_bc[:])
                    nc.scalar.activation(xnb[:], xn[:], ACTF.Copy)
                for fg in range(max(1, DT // 4)):
                    nft = min(4, DT)
                    pt = pt_(pspool, [128, 512], BF16, "ptr")
                    for i in range(nft):
                        ft = fg * 4 + i
                        nc.tensor.matmul(
                            pt[:, i * 128:(i + 1) * 128],
                            xnb[:, ft * 128:(ft + 1) * 128], ident[:],
                            is_transpose=True, skip_group_check=True)
                    nc.vector.tensor_copy(
                        xn_fm[:, fg * 4:fg * 4 + nft,
                              qt * 128:(qt + 1) * 128],
                        pt[:, :nft * 128].rearrange("p (a b) -> p a b",
                                                    a=nft))
            return xn_fm

        def load_bc(pool, src_ap, tag):
            """Broadcast a [D] dram row across 128 partitions."""
            t = tl_(pool, [128, D], F32, tag)
            nc.sync.dma_start(t[:], src_ap[None, :].to_broadcast([128, D]))
            return t

        # ---------------- transformer layers ----------------
        with tc.tile_pool(name="lsb", bufs=2) as sp, \
             tc.tile_pool(name="lw", bufs=2) as wp, \
             tc.tile_pool(name="lps", bufs=2, space="PSUM") as ps, \
             tc.tile_pool(name="ldram", bufs=2, space="DRAM") as dp:
            for l in range(L):
                if cfg.use_ln_affine:
                    g1_bc = load_bc(sp, lnp_d[l, 0], "g1bc")
                    b1_bc = load_bc(sp, lnp_d[l, 1], "b1bc")
                    g2_bc = load_bc(sp, lnp_d[l, 2], "g2bc")
                    b2_bc = load_bc(sp, lnp_d[l, 3], "b2bc")
                else:
                    g1_bc = b1_bc = g2_bc = b2_bc = None

                xn_fm = layer_norm(sp, ps, g1_bc, b1_bc, f"l{l}")

                def wtiles(wdram, tag, width, n):
                    ts = []
                    for k in range(n):
                        t = tl_(wp, [128, width], BF16, tag, f"{tag}_t")
                        nc.sync.dma_start(t[:], wdram[k])
                        ts.append(t)
                    return ts

                def proj_fm(wdram, dst_tag, src_fm):
                    """dst[dout, q] (feature-major) = W.T @ x  -> bf16"""
                    wt = wtiles(wdram, "wproj", D, DT)
                    dst = tl_(sp, [128, DT, 256], BF16, dst_tag)
                    for m in range(DT):
                        pq = pt_(ps, [128, 256], F32, "pmed")
                        for k in range(DT):
                            nc.tensor.matmul(
                                pq[:], wt[k][:, m * 128:(m + 1) * 128],
                                src_fm[:, k, :],
                                start=(k == 0), stop=(k == DT - 1))
                        nc.vector.tensor_copy(dst[:, m, :], pq[:])
                    return dst

                k_fm = proj_fm(wk_d[l], "k_fm", xn_fm)

                # --- K allgather (launched early so it overlaps V/Q) ---
                rg = [[0, 1, 2, 3], [4, 5, 6, 7]]
                single_ag = not os.environ.get("KAGSPLIT")
                if single_ag:
                    aginKV = dp.tile([cfg.AGN], BF16, tag="aginKV",
                                     name="aginKV")
                    agoutKV = dp.tile([4, cfg.AGN], BF16, tag="agoutKV",
                                      name="agoutKV")
                    aginK = aginKV[:cfg.AGK]
                    aginV = aginKV[cfg.AGK:]
                    agoutK = agoutKV[:, :cfg.AGK]
                    agoutV = agoutKV[:, cfg.AGK:]
                else:
                    aginK = dp.tile([cfg.AGK], BF16, tag="aginK",
                                    name="aginK")
                    aginV = dp.tile([cfg.AGK], BF16, tag="aginV",
                                    name="aginV")
                    agoutK = dp.tile([4, cfg.AGK], BF16, tag="agoutK",
                                     name="agoutK")
                    agoutV = dp.tile([4, cfg.AGK], BF16, tag="agoutV",
                                     name="agoutV")
                nc.sync.dma_start(
                    aginK[:].rearrange("(ft fi s t) -> fi ft s t",
                                       ft=DT, fi=128, s=2, t=128),
                    k_fm[:].rearrange("fi ft (s t) -> fi ft s t", s=2))
                if os.environ.get("KNOCC"):
                    for r in range(4):
                        nc.sync.dma_start(agoutK[r], aginK[:])
                elif not single_ag:
                    nc.gpsimd.collective_compute(
                        "AllGather", ALU.bypass, replica_groups=rg,
                        ins=[aginK.opt()], outs=[agoutK.opt()])

                # V token-major [128t, 2qt, D]
                wt_v = wtiles(wv_d[l], "wproj", D, DT)
                v_tm = tl_(sp, [128, 2, D], BF16, "v_tm")
                for qt in range(2):
                    for n in range(D // CW):
                        pv = pt_(ps, [128, CW], F32, "pbig")
                        for k in range(DT):
                            nc.tensor.matmul(
                                pv[:], xn_fm[:, k, qt * 128:(qt + 1) * 128],
                                wt_v[k][:, n * CW:(n + 1) * CW],
                                start=(k == 0), stop=(k == DT - 1))
                        nc.vector.tensor_copy(
                            v_tm[:, qt, n * CW:(n + 1) * CW], pv[:])
                nc.sync.dma_start(
                    aginV[:].rearrange("(s t f) -> t s f", s=2, t=128, f=D),
                    v_tm[:])
                if os.environ.get("KNOCC"):
                    for r in range(4):
                        nc.sync.dma_start(agoutV[r], aginV[:])
                elif single_ag:
                    nc.gpsimd.collective_compute(
                        "AllGather", ALU.bypass, replica_groups=rg,
                        ins=[aginKV.opt()], outs=[agoutKV.opt()])
                else:
                    nc.gpsimd.collective_compute(
                        "AllGather", ALU.bypass, replica_groups=rg,
                        ins=[aginV.opt()], outs=[agoutV.opt()])

                q_fm = proj_fm(wq_d[l], "q_fm", xn_fm)
                k_all = tl_(sp, [128, DT, 8, 128], BF16, "k_all")
                v_all = tl_(sp, [128, 8, D], BF16, "v_all")
                for r in range(4):
                    nc.sync.dma_start(
                        k_all[:, :, 2 * r:2 * r + 2, :],
                        agoutK[r].rearrange(
                            "(ft fi s t) -> fi ft s t", ft=DT, fi=128, s=2,
                            t=128))
                    nc.sync.dma_start(
                        v_all[:, 2 * r:2 * r + 2, :],
                        agoutV[r].rearrange("(s t f) -> t s f", s=2,
                                            t=128, f=D))

                # --- attention ---
                o_fm = tl_(sp, [128, DT, 256], BF16, "o_fm")
                for qt in range(2):
                    nk = 4 if qt == 0 else 8
                    ng = nk // 4
                    qsl = slice(qt * 128, (qt + 1) * 128)
                    for hp in range(NHP):
                        pav = pt_(ps, [128, 128], F32, "pmed")
                        pTs = []
                        for sub in range(2):
                            hh = 2 * hp + sub
                            rows = slice(64 * sub, 64 * sub + 64)
                            probsT = tl_(sp, [128, nk, 128], BF16,
                                         f"probsT{qt}")
                            probs_a = tl_(sp, [128, nk, 128], BF16,
                                          f"probsa{qt}")
                            srow = tl_(sp, [128, ng], F32, "srow")
                            for g in range(ng):
                                sps = pt_(ps, [128, 512], F32, "pbig")
                                if qt == 0:
                                    rhs = k_all[rows, hp, 0:8:2, :]
                                    msk = m0[:]
                                else:
                                    rhs = k_all[rows, hp, 4 * g:4 * g + 4, :]
                                    msk = m1[:, 4 * g:4 * g + 4, :]
                                # additive mask enters via PE accumulation
                                nc.tensor.matmul(
                                    sps[:], ident[:],
                                    msk.rearrange("p a b -> p (a b)"),
                                    start=True, stop=False)
                                nc.tensor.matmul(sps[:], q_fm[rows, hp, qsl],
                                                 rhs, start=False, stop=True)
                                nc.scalar.activation(
                                    probs_a[:, 4 * g:4 * g + 4, :],
                                    sps[:].rearrange("p (a b) -> p a b", a=4),
                                    ACTF.Exp, scale=float(HD ** -0.5),
                                    accum_out=srow[:, g:g + 1])
                            stot = tl_(sp, [128, 1], F32, "stot")
                            nc.vector.tensor_reduce(stot[:], srow[:],
                                                    axis=AX.X, op=ALU.add)
                            sinv = tl_(sp, [128, 1], F32, "sinv")
                            nc.vector.reciprocal(sinv[:], stot[:])
                            pb = tl_(sp, [128, nk, 128], BF16, f"pb{qt}")
                            nc.vector.tensor_scalar_mul(pb[:], probs_a[:],
                                                        sinv[:])
                            for g in range(ng):
                                ptp = pt_(ps, [128, 512], BF16, "ptr")
                                for i in range(4):
                                    nc.tensor.matmul(
                                        ptp[:, i * 128:(i + 1) * 128],
                                        pb[:, 4 * g + i, :], ident[:],
                                        is_transpose=True,
                                        skip_group_check=True)
                                cp = nc.vector if (g + sub) % 2 else nc.scalar
                                if cp is nc.scalar:
                                    nc.scalar.activation(
                                        probsT[:, 4 * g:4 * g + 4, :],
                                        ptp[:].rearrange(
                                            "p (a b) -> p a b", a=4),
                                        ACTF.Copy)
                                else:
                                    nc.vector.tensor_copy(
                                        probsT[:, 4 * g:4 * g + 4, :],
                                        ptp[:].rearrange(
                                            "p (a b) -> p a b", a=4))
                            pTs.append(probsT)
                        # AV: interleave subs for column-group concurrency
                        for i in range(nk):
                            for sub in range(2):
                                hh = 2 * hp + sub
                                rows = slice(64 * sub, 64 * sub + 64)
                                blk = 2 * i if qt == 0 else i
                                tp = (0, 64) if sub == 1 else None
                                nc.tensor.matmul(
                                    pav[rows, :],
                                    v_all[:, blk, 64 * hh:64 * hh + 64],
                                    pTs[sub][:, i, :],
                                    start=(i == 0), stop=(i == nk - 1),
                                    tile_position=tp,
                                    skip_group_check=True)
                        nc.vector.tensor_copy(
                            o_fm[:, hp, qt * 128:(qt + 1) * 128], pav[:])

                # --- Wo + residual ---
                wt_o = wtiles(wo_d[l], "wproj", D, DT)
                if cfg.use_bo:
                    bo_bc = load_bc(sp, bo_d[l], "bobc")
                for qt in range(2):
                    for n in range(D // CW):
                        po = pt_(ps, [128, CW], F32, "pbig")
                        for k in range(DT):
                            nc.tensor.matmul(
                                po[:], o_fm[:, k, qt * 128:(qt + 1) * 128],
                                wt_o[k][:, n * CW:(n + 1) * CW],
                                start=(k == 0), stop=(k == DT - 1))
                        hsl = h[:, qt, n * CW:(n + 1) * CW]
                        nc.vector.tensor_add(hsl, hsl, po[:])
                        if cfg.use_bo:
                            nc.vector.tensor_add(
                                hsl, hsl, bo_bc[:, n * CW:(n + 1) * CW])

                if os.environ.get("KDEBUG"):
                    nc.sync.dma_start(hdbg2[l], h[:])
                    nc.sync.dma_start(odbg[l], o_fm[:])
                # --- FFN ---
                xn2_fm = layer_norm(sp, ps, g2_bc, b2_bc, f"l{l}x2")
                relu_fm = tl_(sp, [128, FT, 256], BF16, "relu")
                for mg in range(MG):
                    w1_t = []
                    for k in range(DT):
                        t = tl_(wp, [128, 512], BF16, "w1", "w1_t")
                        nc.sync.dma_start(t[:], w1_d[l, k, mg])
                        w1_t.append(t)
                    for mi in range(4):
                        m = 4 * mg + mi
                        p1 = pt_(ps, [128, 256], F32, "pmed")
                        for k in range(DT):
                            nc.tensor.matmul(
                                p1[:], w1_t[k][:, mi * 128:(mi + 1) * 128],
                                xn2_fm[:, k, :],
                                start=(k == 0), stop=(k == DT - 1))
                        bias = b1f_sb[:, l, m:m + 1] if cfg.use_b1 else 0.0
                        nc.scalar.activation(relu_fm[:, m, :], p1[:],
                                             ACTF.Relu, bias=bias)
                if os.environ.get("KDEBUG"):
                    nc.sync.dma_start(xdbg[l], xn2_fm[:])
                    nc.sync.dma_start(rdbg[l], relu_fm[:])
                # W2: k-outer, 4 live psums
                if cfg.use_b2:
                    b2r_bc = load_bc(sp, b2_d[l], "b2rbc")
                ND = D // CW
                p2 = [pt_(ps, [128, CW], F32, "pbig") for _ in range(2 * ND)]
                for k2 in range(FT):
                    w2_t = tl_(wp, [128, D], BF16, "w2", "w2_t")
                    nc.sync.dma_start(w2_t[:], w2_d[l, k2])
                    for qt in range(2):
                        for n in range(ND):
                            nc.tensor.matmul(
                                p2[ND * qt + n][:],
                                relu_fm[:, k2, qt * 128:(qt + 1) * 128],
                                w2_t[:, n * CW:(n + 1) * CW],
                                start=(k2 == 0), stop=(k2 == FT - 1))
                for qt in range(2):
                    for n in range(ND):
                        hsl = h[:, qt, n * CW:(n + 1) * CW]
                        nc.vector.tensor_add(hsl, hsl, p2[ND * qt + n][:])
                        if cfg.use_b2:
                            nc.vector.tensor_add(
                                hsl, hsl, b2r_bc[:, n * CW:(n + 1) * CW])
                if os.environ.get("KDEBUG"):
                    nc.sync.dma_start(hdbg[l + 1], h[:])

        # ---------------- LM head + loss ----------------
        with tc.tile_pool(name="fsb", bufs=2) as sp, \
             tc.tile_pool(name="fw", bufs=2) as wp, \
             tc.tile_pool(name="fps", bufs=2, space="PSUM") as ps:
            if cfg.use_lnf_affine:
                gf_bc = load_bc(sp, lnf_d[0], "gfbc")
                bf_bc = load_bc(sp, lnf_d[1], "bfbc")
            else:
                gf_bc = bf_bc = None
            hf_fm = layer_norm(sp, ps, gf_bc, bf_bc, "f")
            if cfg.use_lm_b:
                ones_t = tl_(sp, [1, 256], BF16, "ones")
                nc.vector.memset(ones_t[:], 1.0)
                lmb_sb = tl_(sp, [1, V], BF16, "lmb_sb")
                nc.sync.dma_start(lmb_sb[:], lmb_d[None, :])
            expacc = tl_(sp, [128, 2, VG * NVG], F32, "expacc")
            for vg in range(NVG):
                lm_t = []
                for k in range(DT):
                    t = tl_(wp, [128, VC * VG], BF16, "lm", "lm_t")
                    nc.sync.dma_start(
                        t[:], lm_d[k][:, vg * VC * VG:(vg + 1) * VC * VG])
                    lm_t.append(t)
                for c in range(VG):
                    col0 = vg * VG * VC + c * VC
                    for qt in range(2):
                        pl = pt_(ps, [128, VC], F32, "pbig")
                        for k in range(DT):
                            nc.tensor.matmul(
                                pl[:], hf_fm[:, k, qt * 128:(qt + 1) * 128],
                                lm_t[k][:, c * VC:(c + 1) * VC],
                                start=(k == 0),
                                stop=(k == DT - 1 and not cfg.use_lm_b))
                        if cfg.use_lm_b:
                            nc.tensor.matmul(
                                pl[:], ones_t[:, qt * 128:(qt + 1) * 128],
                                lmb_sb[:, col0:col0 + VC],
                                start=False, stop=True)
                        lsb = tl_(sp, [128, VC], F32, "lsb")
                        nc.vector.tensor_copy(lsb[:], pl[:])
                        nc.sync.dma_start(
                            logits_out[qt * 128:(qt + 1) * 128,
                                       col0:col0 + VC], lsb[:])
                        esc = tl_(sp, [128, VC], BF16, "esc")
                        nc.scalar.activation(
                            esc[:], pl[:], ACTF.Exp,
                            accum_out=expacc[:, qt,
                                             vg * VG + c:vg * VG + c + 1])
            # lse and loss terms
            lt_all = tl_(sp, [128, 2], F32, "lt")
            for qt in range(2):
                stot = tl_(sp, [128, 1], F32, "fstot")
                nc.vector.tensor_reduce(stot[:], expacc[:, qt, :], axis=AX.X,
                                        op=ALU.add)
                lse = tl_(sp, [128, 1], F32, "flse")
                nc.scalar.activation(lse[:], stot[:], ACTF.Ln)
                # gather target logits back from DRAM logits
                grow = tl_(sp, [128, 1, 512], F32, "grow")
                src = logits_out[qt * 128:(qt + 1) * 128, :].rearrange(
                    "a b -> (a b)").rearrange("(n e) -> n e", e=512)
                gidx = tl_(sp, [128, 8], I16, "gidx")
                nc.sync.dma_start(gidx[:], tgt_idx[:, qt * 8:(qt + 1) * 8])
                nc.gpsimd.dma_gather(grow[:], src, gidx[:], num_idxs=128,
                                     num_idxs_reg=128, elem_size=512)
                tm_sb = tl_(sp, [128, 512], F32, "tm_sb")
                nc.sync.dma_start(tm_sb[:], tmask[:, qt, :])
                prod = tl_(sp, [128, 512], F32, "prod")
                nc.vector.tensor_mul(prod[:], grow[:, 0, :], tm_sb[:])
                tlg = tl_(sp, [128, 1], F32, "tl")
                nc.vector.tensor_reduce(tlg[:], prod[:], axis=AX.X, op=ALU.add)
                nc.vector.tensor_sub(lt_all[:, qt:qt + 1], lse[:], tlg[:])
            nc.sync.dma_start(loss_out[:], lt_all[:])

    nc.compile()
    return nc


# ---------------------------------------------------------------------------
# host-side prep / run
# ---------------------------------------------------------------------------

def _prep_shared(cfg, tok_emb, Wq, Wk, Wv, Wo, W1, W2, lm_W, lm_b,
                 ln1_g, ln1_b, ln2_g, ln2_b, lnf_g, lnf_b, bo, b1, b2):
    D, L, FF, V, DT, FT = cfg.D, cfg.L, cfg.FF, cfg.V, cfg.DT, cfg.FT
    MG = FT // 4
    sh = {
        "tok_emb": np.ascontiguousarray(tok_emb, dtype=np.float32),
        "wq": np.ascontiguousarray(Wq.reshape(L, DT, 128, D).astype(BF)),
        "wk": np.ascontiguousarray(Wk.reshape(L, DT, 128, D).astype(BF)),
        "wv": np.ascontiguousarray(Wv.reshape(L, DT, 128, D).astype(BF)),
        "wo": np.ascontiguousarray(Wo.reshape(L, DT, 128, D).astype(BF)),
        "w1": np.ascontiguousarray(
            W1.reshape(L, DT, 128, MG, 512).transpose(0, 1, 3, 2, 4)
            .astype(BF)),
        "w2": np.ascontiguousarray(W2.reshape(L, FT, 128, D).astype(BF)),
        "lmw": np.ascontiguousarray(lm_W.reshape(DT, 128, V).astype(BF)),
    }
    if cfg.use_ln_affine:
        sh["lnp"] = np.ascontiguousarray(
            np.stack([ln1_g, ln1_b, ln2_g, ln2_b], axis=1), dtype=np.float32)
    if cfg.use_lnf_affine:
        sh["lnf"] = np.ascontiguousarray(
            np.stack([lnf_g, lnf_b]), dtype=np.float32)
    if cfg.use_b1:
        sh["b1f"] = np.ascontiguousarray(
            b1.reshape(L, FT, 128).transpose(2, 0, 1), dtype=np.float32)
    if cfg.use_bo:
        sh["bo_in"] = np.ascontiguousarray(bo, dtype=np.float32)
    if cfg.use_b2:
        sh["b2_in"] = np.ascontiguousarray(b2, dtype=np.float32)
    if cfg.use_lm_b:
        sh["lmb"] = np.ascontiguousarray(lm_b.astype(BF))
    return sh


def _wrap16(idx, width):
    """[n] -> [128, n//16]: 16-partition wrapped layout (idx i at
    [i%16, i//16]), replicated to 128 partitions."""
    n = idx.shape[0]
    out = np.zeros((16, width), dtype=np.int16)
    for i in range(n):
        out[i % 16, i // 16] = idx[i]
    return np.ascontiguousarray(np.tile(out, (8, 1)))


def _prep_core(cfg, c, x, targets, pos_emb):
    D, V = cfg.D, cfg.V
    b, r = c // 4, c % 4
    j0, j1 = r, 7 - r
    rows0 = slice(128 * j0, 128 * j0 + 128)
    rows1 = slice(128 * j1, 128 * j1 + 128)
    toks = np.concatenate([x[b, rows0], x[b, rows1]]).astype(np.int64)
    tgts = np.concatenate([targets[b, rows0], targets[b, rows1]]).astype(np.int64)
    pos = np.concatenate([pos_emb[rows0], pos_emb[rows1]])  # [256, D]

    per = {
        "tok_idx": _wrap16(toks.astype(np.int16), 16),
        "pos_sl": np.ascontiguousarray(
            pos.reshape(2, 128, D).transpose(1, 0, 2), dtype=np.float32),
    }
    # additive masks (0 = attend, -1e9 = masked); scores in AG block order
    p = np.arange(128)[:, None]
    xk = np.arange(128)[None, :]
    m0 = np.zeros((128, 4, 128), dtype=np.float32)
    for i in range(4):
        m0[:, i, :] = np.where(128 * i + xk <= 128 * j0 + p, 0.0, -1e9)
    m1 = np.zeros((128, 8, 128), dtype=np.float32)
    for bq in range(8):
        j = bq // 2 if bq % 2 == 0 else 7 - (bq - 1) // 2
        m1[:, bq, :] = np.where(128 * j + xk <= 128 * j1 + p, 0.0, -1e9)
    per["mask0"] = np.ascontiguousarray(m0.astype(BF))
    per["mask1"] = np.ascontiguousarray(m1.astype(BF))
    # target gather idx + mask
    tgt16 = np.zeros((128, 16), dtype=np.int16)
    tm = np.zeros((128, 2, 512), dtype=np.float32)
    for qt in range(2):
        tloc = tgts[qt * 128:(qt + 1) * 128]
        flat = np.arange(128) * V + tloc
        row512 = (flat // 512).astype(np.int16)
        within = flat % 512
        tgt16[:, qt * 8:(qt + 1) * 8] = _wrap16(row512, 8)
        tm[np.arange(128), qt, within] = 1.0
    per["tgt_idx"] = tgt16
    per["tmask"] = tm
    return per


_BUILD_CACHE = {}


def _get_nc(cfg):
    key = (cfg.D, cfg.L, cfg.FF, cfg.V, cfg.VC, cfg.VG, cfg.use_ln_affine,
           cfg.use_bo, cfg.use_b1, cfg.use_b2, cfg.use_lnf_affine,
           cfg.use_lm_b)
    if key not in _BUILD_CACHE:
        _BUILD_CACHE[key] = build(cfg)
    return _BUILD_CACHE[key]


def prepare(cfg, x, targets, tok_emb, pos_emb, Wq, Wk, Wv, Wo, bo, W1, b1,
            W2, b2, ln1_g, ln1_b, ln2_g, ln2_b, lnf_g, lnf_b, lm_W, lm_b):
    """Returns (nc, in_maps)."""
    x = np.asarray(x)
    targets = np.asarray(targets)
    args = [np.asarray(a, dtype=np.float32) for a in
            (tok_emb, pos_emb, Wq, Wk, Wv, Wo, bo, W1, b1, W2, b2,
             ln1_g, ln1_b, ln2_g, ln2_b, lnf_g, lnf_b, lm_W, lm_b)]
    (tok_emb, pos_emb, Wq, Wk, Wv, Wo, bo, W1, b1, W2, b2,
     ln1_g, ln1_b, ln2_g, ln2_b, lnf_g, lnf_b, lm_W, lm_b) = args

    cfg.use_ln_affine = not (np.all(ln1_g == 1) and np.all(ln1_b == 0)
                             and np.all(ln2_g == 1) and np.all(ln2_b == 0))
    cfg.use_lnf_affine = not (np.all(lnf_g == 1) and np.all(lnf_b == 0))
    cfg.use_bo = bool(np.any(bo))
    cfg.use_b1 = bool(np.any(b1))
    cfg.use_b2 = bool(np.any(b2))
    cfg.use_lm_b = bool(np.any(lm_b))

    nc = _get_nc(cfg)
    sh = _prep_shared(cfg, tok_emb, Wq, Wk, Wv, Wo, W1, W2, lm_W, lm_b,
                      ln1_g, ln1_b, ln2_g, ln2_b, lnf_g, lnf_b, bo, b1, b2)
    in_maps = []
    for c in range(N_CORES):
        m = dict(sh)
        m.update(_prep_core(cfg, c, x, targets, pos_emb))
        in_maps.append(m)
    return nc, in_maps


def assemble(cfg, results):
    B, T, V = cfg.B, cfg.T, cfg.V
    logits = np.zeros((B, T, V), dtype=np.float32)
    loss_sum = 0.0
    for c in range(N_CORES):
        b, r = c // 4, c % 4
        lo = results[c]["logits_out"]
        logits[b, 128 * r:128 * r + 128] = lo[:128]
        logits[b, 128 * (7 - r):128 * (7 - r) + 128] = lo[128:]
        loss_sum += float(np.asarray(results[c]["loss_terms"],
                                     dtype=np.float64).sum())
    loss = np.float32(loss_sum / (B * T))
    return logits, loss


def kernel(x, targets, tok_emb, pos_emb, Wq, Wk, Wv, Wo, bo, W1, b1, W2, b2,
           ln1_g, ln1_b, ln2_g, ln2_b, lnf_g, lnf_b, lm_W, lm_b):
    cfg = Cfg()
    nc, in_maps = prepare(cfg, x, targets, tok_emb, pos_emb, Wq, Wk, Wv, Wo,
                          bo, W1, b1, W2, b2, ln1_g, ln1_b, ln2_g, ln2_b,
                          lnf_g, lnf_b, lm_W, lm_b)
    res = bass_utils.run_bass_kernel_spmd(
        nc, in_maps, core_ids=list(range(N_CORES)), trace=False)
    return assemble(cfg, res.results)


# revision 24
# speedup vs baseline: 5.4453x; 5.4453x over previous
"""Trainium2 Bass kernel for an 8-layer GPT forward pass + CE loss.

Distribution (8 NeuronCores): token-sharded. Core c handles sequence b=c//4
(of B=2) and, within that sequence's 1024 tokens, two 128-token half-chunks
hc_r and hc_{7-r} (r=c%4) — the symmetric pairing balances causal-attention
cost across cores. Per layer, each core computes LN/QKV/FFN for its own 256
tokens only; K and V chunks are exchanged within each 4-core group via one
AllGather per layer. The LM head and the per-token CE terms are computed
token-sharded as well (no vocab sharding, so the softmax over V=32000 is
core-local). The host only shards inputs, concatenates the logit chunks and
averages the per-token loss terms.

Matmul operands are cast to bf16 (fp32 accumulation in PSUM); LN, softmax,
residual stream and logits stay fp32.
"""

import os
import sys
from contextlib import ExitStack

import numpy as np
import ml_dtypes

for _p in ("/opt/trn_rl_repo", "/root/.axon_site/_ro/trn_rl_repo"):
    if os.path.isdir(_p) and _p not in sys.path:
        sys.path.insert(0, _p)

import concourse.bass as bass
import concourse.mybir as mybir
import concourse.tile as tile
from concourse import bacc
import concourse.bass_utils as bass_utils
from concourse.masks import make_identity

F32 = mybir.dt.float32
BF16 = mybir.dt.bfloat16
I16 = mybir.dt.int16
AX = mybir.AxisListType
ALU = mybir.AluOpType
ACTF = mybir.ActivationFunctionType

BF = ml_dtypes.bfloat16

N_CORES = 8
EPS = 1e-5

# sbuf tile tag -> bufs (must be consistent per tag)
TB = {
    "xn_fm": 2, "st6": 2, "agg": 2, "sd": 2, "rinv": 2, "xn": 2, "xnb": 2,
    "wproj": 12, "q_fm": 1, "k_fm": 1, "v_tm": 1, "o_fm": 1,
    "k_all": 1, "v_all": 1,
    "probsT0": 2, "probsT1": 2, "probsa0": 2, "probsa1": 2, "srow": 2, "stot": 2, "sinv": 2,
    "pb0": 2, "pb1": 2,
    "w1": 10, "relu": 1, "w2": 3,
    "g1bc": 2, "b1bc": 2, "g2bc": 2, "b2bc": 2, "bobc": 2, "b2rbc": 2,
    "gfbc": 1, "bfbc": 1,
    "lm": 12, "lsb": 4, "esc": 2, "expacc": 1, "ones": 1, "lmb_sb": 1,
    "fstot": 2, "flse": 2, "grow": 2, "gidx": 2, "tm_sb": 2, "tl": 2,
    "prod": 2, "lt": 1,
}
PB = {"pbig": 4, "pmed": 2, "ptr": 2}


class Cfg:
    def __init__(self, D=1024, H=16, L=8, FF=4096, V=32000, VC=500, VG=8):
        self.D, self.H, self.L, self.FF, self.V = D, H, L, FF, V
        self.HD = 64
        assert D == self.HD * H
        self.DT = D // 128          # D tiles
        self.FT = FF // 128         # FF tiles
        self.VC = VC                # vocab chunk (psum free dim)
        self.VG = VG                # chunks per vocab group
        self.NVG = V // (VC * VG)   # vocab groups
        assert V == VC * VG * self.NVG
        assert (128 * V) % 512 == 0
        self.T = 1024               # tokens per sequence (8 half-chunks of 128)
        self.B = 2
        self.AGK = 256 * D          # K_fm part of allgather payload (elems)
        self.AGN = 2 * 256 * D      # total allgather payload per core (elems)
        # flags set from input values at build time
        self.use_ln_affine = False
        self.use_bo = False
        self.use_b1 = False
        self.use_b2 = False
        self.use_lnf_affine = False
        self.use_lm_b = False


def build(cfg: Cfg):
    nc = bacc.Bacc("TRN2", target_bir_lowering=False, debug=False,
                   num_devices=N_CORES)
    D, H, L, FF, V = cfg.D, cfg.H, cfg.L, cfg.FF, cfg.V
    DT, FT, VC, VG, NVG = cfg.DT, cfg.FT, cfg.VC, cfg.VG, cfg.NVG
    HD = cfg.HD
    NHP = H // 2  # head pairs
    MG = FT // 4  # W1 m-groups (4 m-tiles each)
    CW = min(512, D)  # output chunk width

    dram = lambda name, shape, dt, kind="ExternalInput": nc.dram_tensor(
        name, shape, dt, kind=kind).ap()

    tok_emb = dram("tok_emb", [V, D], F32)
    pos_sl = dram("pos_sl", [128, 2, D], F32)
    tok_idx = dram("tok_idx", [128, 16], I16)
    wq_d = dram("wq", [L, DT, 128, D], BF16)
    wk_d = dram("wk", [L, DT, 128, D], BF16)
    wv_d = dram("wv", [L, DT, 128, D], BF16)
    wo_d = dram("wo", [L, DT, 128, D], BF16)
    w1_d = dram("w1", [L, DT, MG, 128, 512], BF16)   # m-grouped tiles
    w2_d = dram("w2", [L, FT, 128, D], BF16)
    lm_d = dram("lmw", [DT, 128, V], BF16)
    mask0 = dram("mask0", [128, 4, 128], BF16)
    mask1 = dram("mask1", [128, 8, 128], BF16)
    tmask = dram("tmask", [128, 2, 512], F32)
    tgt_idx = dram("tgt_idx", [128, 16], I16)
    if cfg.use_ln_affine:
        lnp_d = dram("lnp", [L, 4, D], F32)  # ln1_g, ln1_b, ln2_g, ln2_b
    if cfg.use_lnf_affine:
        lnf_d = dram("lnf", [2, D], F32)
    if cfg.use_b1:
        b1f_d = dram("b1f", [128, L, FT], F32)
    if cfg.use_bo:
        bo_d = dram("bo_in", [L, D], F32)
    if cfg.use_b2:
        b2_d = dram("b2_in", [L, D], F32)
    if cfg.use_lm_b:
        lmb_d = dram("lmb", [V], BF16)

    logits_out = dram("logits_out", [256, V], F32, kind="ExternalOutput")
    if os.environ.get("KDEBUG"):
        hdbg = dram("h_dbg", [L + 1, 128, 2, D], F32, kind="ExternalOutput")
        hdbg2 = dram("h_dbg2", [L, 128, 2, D], F32, kind="ExternalOutput")
        odbg = dram("o_dbg", [L, 128, DT, 256], BF16, kind="ExternalOutput")
        xdbg = dram("x_dbg", [L, 128, DT, 256], BF16, kind="ExternalOutput")
        rdbg = dram("r_dbg", [L, 128, FT, 256], BF16, kind="ExternalOutput")
    loss_out = dram("loss_terms", [128, 2], F32, kind="ExternalOutput")

    with tile.TileContext(nc) as tc, ExitStack() as ctx:
        pers = ctx.enter_context(tc.tile_pool(name="pers", bufs=1))

        ident = pers.tile([128, 128], BF16, name="ident")
        make_identity(nc, ident[:])
        m0 = pers.tile([128, 4, 128], BF16, name="m0")
        nc.sync.dma_start(m0[:], mask0[:])
        m1 = pers.tile([128, 8, 128], BF16, name="m1")
        nc.sync.dma_start(m1[:], mask1[:])
        h = pers.tile([128, 2, D], F32, name="h")
        eps_t = pers.tile([128, 1], F32, name="eps_t")
        nc.gpsimd.memset(eps_t[:], EPS)
        if cfg.use_b1:
            b1f_sb = pers.tile([128, L, FT], F32, name="b1f_sb")
            nc.sync.dma_start(b1f_sb[:], b1f_d[:])

        # --- embedding: h = tok_emb[x] + pos ---
        idx_sb = pers.tile([128, 16], I16, name="idx_sb")
        nc.sync.dma_start(idx_sb[:], tok_idx[:])
        nc.gpsimd.dma_gather(h[:], tok_emb[:], idx_sb[:],
                             num_idxs=256, num_idxs_reg=256, elem_size=D)
        pos_sb = pers.tile([128, 2, D], F32, name="pos_sb")
        nc.sync.dma_start(pos_sb[:], pos_sl[:])
        nc.vector.tensor_add(h[:], h[:], pos_sb[:])
        if os.environ.get("KDEBUG"):
            nc.sync.dma_start(hdbg[0], h[:])

        bnb = min(512, D)  # bn_stats free-dim limit

        def tl_(pool, shape, dt, tag, name=None):
            return pool.tile(shape, dt, tag=tag, bufs=TB[tag],
                             name=name or tag)

        def pt_(pool, shape, dt, tag):
            return pool.tile(shape, dt, tag=tag, bufs=PB[tag], name=tag)

        def layer_norm(pool, pspool, g_bc, b_bc, name):
            """LN over h (token-major) -> xn_fm bf16 [128, DT, 256]."""
            xn_fm = tl_(pool, [128, DT, 256], BF16, "xn_fm", f"xnfm_{name}")
            for qt in range(2):
                hqt = h[:, qt, :]
                st6 = tl_(pool, [128, D // bnb, 6], F32, "st6")
                for a in range(D // bnb):
                    nc.vector.bn_stats(st6[:, a, :],
                                       hqt[:, a * bnb:(a + 1) * bnb])
                agg = tl_(pool, [128, 2], F32, "agg")
                nc.vector.bn_aggr(agg[:], st6[:])
                sd = tl_(pool, [128, 1], F32, "sd")
                nc.scalar.activation(sd[:], agg[:, 1:2], ACTF.Sqrt, bias=eps_t[:])
                rinv = tl_(pool, [128, 1], F32, "rinv")
                nc.vector.reciprocal(rinv[:], sd[:])
                xnb = tl_(pool, [128, D], BF16, "xnb")
                if g_bc is None and b_bc is None:
                    # fused: (h - mean) * rinv straight to bf16
                    nc.vector.tensor_scalar(xnb[:], hqt, agg[:, 0:1], rinv[:],
                                            op0=ALU.subtract, op1=ALU.mult)
                else:
                    xn = tl_(pool, [128, D], F32, "xn")
                    nc.vector.tensor_scalar(xn[:], hqt, agg[:, 0:1], rinv[:],
                                            op0=ALU.subtract, op1=ALU.mult)
                    if g_bc is not None:
                        nc.vector.tensor_mul(xn[:], xn[:], g_bc[:])
                    if b_bc is not None:
                        nc.vector.tensor_add(xn[:], xn[:], b_bc[:])
                    nc.scalar.activation(xnb[:], xn[:], ACTF.Copy)
                for fg in range(max(1, DT // 4)):
                    nft = min(4, DT)
                    pt = pt_(pspool, [128, 512], BF16, "ptr")
                    for i in range(nft):
                        ft = fg * 4 + i
                        nc.tensor.matmul(
                            pt[:, i * 128:(i + 1) * 128],
                            xnb[:, ft * 128:(ft + 1) * 128], ident[:],
                            is_transpose=True, skip_group_check=True)
                    nc.vector.tensor_copy(
                        xn_fm[:, fg * 4:fg * 4 + nft,
                              qt * 128:(qt + 1) * 128],
                        pt[:, :nft * 128].rearrange("p (a b) -> p a b",
                                                    a=nft))
            return xn_fm

        def load_bc(pool, src_ap, tag):
            """Broadcast a [D] dram row across 128 partitions."""
            t = tl_(pool, [128, D], F32, tag)
            nc.sync.dma_start(t[:], src_ap[None, :].to_broadcast([128, D]))
            return t

        # ---------------- transformer layers ----------------
        with tc.tile_pool(name="lsb", bufs=2) as sp, \
             tc.tile_pool(name="lw", bufs=2) as wp, \
             tc.tile_pool(name="lps", bufs=2, space="PSUM") as ps, \
             tc.tile_pool(name="ldram", bufs=2, space="DRAM") as dp:
            for l in range(L):
                if cfg.use_ln_affine:
                    g1_bc = load_bc(sp, lnp_d[l, 0], "g1bc")
                    b1_bc = load_bc(sp, lnp_d[l, 1], "b1bc")
                    g2_bc = load_bc(sp, lnp_d[l, 2], "g2bc")
                    b2_bc = load_bc(sp, lnp_d[l, 3], "b2bc")
                else:
                    g1_bc = b1_bc = g2_bc = b2_bc = None

                xn_fm = layer_norm(sp, ps, g1_bc, b1_bc, f"l{l}")

                def wtiles(wdram, tag, width, n):
                    ts = []
                    for k in range(n):
                        t = tl_(wp, [128, width], BF16, tag, f"{tag}_t")
                        nc.sync.dma_start(t[:], wdram[k])
                        ts.append(t)
                    return ts

                def proj_fm(wdram, dst_tag, src_fm):
                    """dst[dout, q] (feature-major) = W.T @ x  -> bf16"""
                    wt = wtiles(wdram, "wproj", D, DT)
                    dst = tl_(sp, [128, DT, 256], BF16, dst_tag)
                    for m in range(DT):
                        pq = pt_(ps, [128, 256], F32, "pmed")
                        for k in range(DT):
                            nc.tensor.matmul(
                                pq[:], wt[k][:, m * 128:(m + 1) * 128],
                                src_fm[:, k, :],
                                start=(k == 0), stop=(k == DT - 1))
                        nc.vector.tensor_copy(dst[:, m, :], pq[:])
                    return dst

                k_fm = proj_fm(wk_d[l], "k_fm", xn_fm)

                # --- K allgather (launched early so it overlaps V/Q) ---
                rg = [[0, 1, 2, 3], [4, 5, 6, 7]]
                single_ag = not os.environ.get("KAGSPLIT")
                if single_ag:
                    aginKV = dp.tile([cfg.AGN], BF16, tag="aginKV",
                                     name="aginKV")
                    agoutKV = dp.tile([4, cfg.AGN], BF16, tag="agoutKV",
                                      name="agoutKV")
                    aginK = aginKV[:cfg.AGK]
                    aginV = aginKV[cfg.AGK:]
                    agoutK = agoutKV[:, :cfg.AGK]
                    agoutV = agoutKV[:, cfg.AGK:]
                else:
                    aginK = dp.tile([cfg.AGK], BF16, tag="aginK",
                                    name="aginK")
                    aginV = dp.tile([cfg.AGK], BF16, tag="aginV",
                                    name="aginV")
                    agoutK = dp.tile([4, cfg.AGK], BF16, tag="agoutK",
                                     name="agoutK")
                    agoutV = dp.tile([4, cfg.AGK], BF16, tag="agoutV",
                                     name="agoutV")
                nc.sync.dma_start(
                    aginK[:].rearrange("(ft fi s t) -> fi ft s t",
                                       ft=DT, fi=128, s=2, t=128),
                    k_fm[:].rearrange("fi ft (s t) -> fi ft s t", s=2))
                if os.environ.get("KNOCC"):
                    for r in range(4):
                        nc.sync.dma_start(agoutK[r], aginK[:])
                elif not single_ag:
                    nc.gpsimd.collective_compute(
                        "AllGather", ALU.bypass, replica_groups=rg,
                        ins=[aginK.opt()], outs=[agoutK.opt()])

                # V token-major [128t, 2qt, D]
                wt_v = wtiles(wv_d[l], "wproj", D, DT)
                v_tm = tl_(sp, [128, 2, D], BF16, "v_tm")
                for qt in range(2):
                    for n in range(D // CW):
                        pv = pt_(ps, [128, CW], F32, "pbig")
                        for k in range(DT):
                            nc.tensor.matmul(
                                pv[:], xn_fm[:, k, qt * 128:(qt + 1) * 128],
                                wt_v[k][:, n * CW:(n + 1) * CW],
                                start=(k == 0), stop=(k == DT - 1))
                        nc.vector.tensor_copy(
                            v_tm[:, qt, n * CW:(n + 1) * CW], pv[:])
                nc.sync.dma_start(
                    aginV[:].rearrange("(s t f) -> t s f", s=2, t=128, f=D),
                    v_tm[:])
                if os.environ.get("KNOCC"):
                    for r in range(4):
                        nc.sync.dma_start(agoutV[r], aginV[:])
                elif single_ag:
                    nc.gpsimd.collective_compute(
                        "AllGather", ALU.bypass, replica_groups=rg,
                        ins=[aginKV.opt()], outs=[agoutKV.opt()])
                else:
                    nc.gpsimd.collective_compute(
                        "AllGather", ALU.bypass, replica_groups=rg,
                        ins=[aginV.opt()], outs=[agoutV.opt()])

                q_fm = proj_fm(wq_d[l], "q_fm", xn_fm)
                k_all = tl_(sp, [128, DT, 8, 128], BF16, "k_all")
                v_all = tl_(sp, [128, 8, D], BF16, "v_all")
                for r in range(4):
                    nc.sync.dma_start(
                        k_all[:, :, 2 * r:2 * r + 2, :],
                        agoutK[r].rearrange(
                            "(ft fi s t) -> fi ft s t", ft=DT, fi=128, s=2,
                            t=128))
                    nc.sync.dma_start(
                        v_all[:, 2 * r:2 * r + 2, :],
                        agoutV[r].rearrange("(s t f) -> t s f", s=2,
                                            t=128, f=D))

                # --- attention ---
                o_fm = tl_(sp, [128, DT, 256], BF16, "o_fm")
                for qt in range(2):
                    nk = 4 if qt == 0 else 8
                    ng = nk // 4
                    qsl = slice(qt * 128, (qt + 1) * 128)
                    for hp in range(NHP):
                        pav = pt_(ps, [128, 128], F32, "pmed")
                        pTs = []
                        for sub in range(2):
                            hh = 2 * hp + sub
                            rows = slice(64 * sub, 64 * sub + 64)
                            probsT = tl_(sp, [128, nk, 128], BF16,
                                         f"probsT{qt}")
                            probs_a = tl_(sp, [128, nk, 128], BF16,
                                          f"probsa{qt}")
                            srow = tl_(sp, [128, ng], F32, "srow")
                            for g in range(ng):
                                sps = pt_(ps, [128, 512], F32, "pbig")
                                if qt == 0:
                                    rhs = k_all[rows, hp, 0:8:2, :]
                                    msk = m0[:]
                                else:
                                    rhs = k_all[rows, hp, 4 * g:4 * g + 4, :]
                                    msk = m1[:, 4 * g:4 * g + 4, :]
                                # additive mask enters via PE accumulation
                                nc.tensor.matmul(
                                    sps[:], ident[:],
                                    msk.rearrange("p a b -> p (a b)"),
                                    start=True, stop=False)
                                nc.tensor.matmul(sps[:], q_fm[rows, hp, qsl],
                                                 rhs, start=False, stop=True)
                                nc.scalar.activation(
                                    probs_a[:, 4 * g:4 * g + 4, :],
                                    sps[:].rearrange("p (a b) -> p a b", a=4),
                                    ACTF.Exp, scale=float(HD ** -0.5),
                                    accum_out=srow[:, g:g + 1])
                            stot = tl_(sp, [128, 1], F32, "stot")
                            nc.vector.tensor_reduce(stot[:], srow[:],
                                                    axis=AX.X, op=ALU.add)
                            sinv = tl_(sp, [128, 1], F32, "sinv")
                            nc.vector.reciprocal(sinv[:], stot[:])
                            pb = tl_(sp, [128, nk, 128], BF16, f"pb{qt}")
                            nc.vector.tensor_scalar_mul(pb[:], probs_a[:],
                                                        sinv[:])
                            for g in range(ng):
                                ptp = pt_(ps, [128, 512], BF16, "ptr")
                                for i in range(4):
                                    nc.tensor.matmul(
                                        ptp[:, i * 128:(i + 1) * 128],
                                        pb[:, 4 * g + i, :], ident[:],
                                        is_transpose=True,
                                        skip_group_check=True)
                                cp = nc.vector if (g + sub) % 2 else nc.scalar
                                if cp is nc.scalar:
                                    nc.scalar.activation(
                                        probsT[:, 4 * g:4 * g + 4, :],
                                        ptp[:].rearrange(
                                            "p (a b) -> p a b", a=4),
                                        ACTF.Copy)
                                else:
                                    nc.vector.tensor_copy(
                                        probsT[:, 4 * g:4 * g + 4, :],
                                        ptp[:].rearrange(
                                            "p (a b) -> p a b", a=4))
                            pTs.append(probsT)
                        # AV: interleave subs for column-group concurrency
                        for i in range(nk):
                            for sub in range(2):
                                hh = 2 * hp + sub
                                rows = slice(64 * sub, 64 * sub + 64)
                                blk = 2 * i if qt == 0 else i
                                tp = (0, 64) if sub == 1 else None
                                nc.tensor.matmul(
                                    pav[rows, :],
                                    v_all[:, blk, 64 * hh:64 * hh + 64],
                                    pTs[sub][:, i, :],
                                    start=(i == 0), stop=(i == nk - 1),
                                    tile_position=tp,
                                    skip_group_check=True)
                        nc.vector.tensor_copy(
                            o_fm[:, hp, qt * 128:(qt + 1) * 128], pav[:])

                # --- Wo + residual ---
                wt_o = wtiles(wo_d[l], "wproj", D, DT)
                if cfg.use_bo:
                    bo_bc = load_bc(sp, bo_d[l], "bobc")
                for qt in range(2):
                    for n in range(D // CW):
                        po = pt_(ps, [128, CW], F32, "pbig")
                        for k in range(DT):
                            nc.tensor.matmul(
                                po[:], o_fm[:, k, qt * 128:(qt + 1) * 128],
                                wt_o[k][:, n * CW:(n + 1) * CW],
                                start=(k == 0), stop=(k == DT - 1))
                        hsl = h[:, qt, n * CW:(n + 1) * CW]
                        nc.vector.tensor_add(hsl, hsl, po[:])
                        if cfg.use_bo:
                            nc.vector.tensor_add(
                                hsl, hsl, bo_bc[:, n * CW:(n + 1) * CW])

                if os.environ.get("KDEBUG"):
                    nc.sync.dma_start(hdbg2[l], h[:])
                    nc.sync.dma_start(odbg[l], o_fm[:])
                # --- FFN ---
                xn2_fm = layer_norm(sp, ps, g2_bc, b2_bc, f"l{l}x2")
                relu_fm = tl_(sp, [128, FT, 256], BF16, "relu")
                for mg in range(MG):
                    w1_t = []
                    for k in range(DT):
                        t = tl_(wp, [128, 512], BF16, "w1", "w1_t")
                        nc.sync.dma_start(t[:], w1_d[l, k, mg])
                        w1_t.append(t)
                    for mi in range(4):
                        m = 4 * mg + mi
                        p1 = pt_(ps, [128, 256], F32, "pmed")
                        for k in range(DT):
                            nc.tensor.matmul(
                                p1[:], w1_t[k][:, mi * 128:(mi + 1) * 128],
                                xn2_fm[:, k, :],
                                start=(k == 0), stop=(k == DT - 1))
                        bias = b1f_sb[:, l, m:m + 1] if cfg.use_b1 else 0.0
                        nc.scalar.activation(relu_fm[:, m, :], p1[:],
                                             ACTF.Relu, bias=bias)
                if os.environ.get("KDEBUG"):
                    nc.sync.dma_start(xdbg[l], xn2_fm[:])
                    nc.sync.dma_start(rdbg[l], relu_fm[:])
                # W2: k-outer, 4 live psums
                if cfg.use_b2:
                    b2r_bc = load_bc(sp, b2_d[l], "b2rbc")
                ND = D // CW
                p2 = [pt_(ps, [128, CW], F32, "pbig") for _ in range(2 * ND)]
                for k2 in range(FT):
                    w2_t = tl_(wp, [128, D], BF16, "w2", "w2_t")
                    nc.sync.dma_start(w2_t[:], w2_d[l, k2])
                    for qt in range(2):
                        for n in range(ND):
                            nc.tensor.matmul(
                                p2[ND * qt + n][:],
                                relu_fm[:, k2, qt * 128:(qt + 1) * 128],
                                w2_t[:, n * CW:(n + 1) * CW],
                                start=(k2 == 0), stop=(k2 == FT - 1))
                for qt in range(2):
                    for n in range(ND):
                        hsl = h[:, qt, n * CW:(n + 1) * CW]
                        nc.vector.tensor_add(hsl, hsl, p2[ND * qt + n][:])
                        if cfg.use_b2:
                            nc.vector.tensor_add(
                                hsl, hsl, b2r_bc[:, n * CW:(n + 1) * CW])
                if os.environ.get("KDEBUG"):
                    nc.sync.dma_start(hdbg[l + 1], h[:])

        # ---------------- LM head + loss ----------------
        with tc.tile_pool(name="fsb", bufs=2) as sp, \
             tc.tile_pool(name="fw", bufs=2) as wp, \
             tc.tile_pool(name="fps", bufs=2, space="PSUM") as ps:
            if cfg.use_lnf_affine:
                gf_bc = load_bc(sp, lnf_d[0], "gfbc")
                bf_bc = load_bc(sp, lnf_d[1], "bfbc")
            else:
                gf_bc = bf_bc = None
            hf_fm = layer_norm(sp, ps, gf_bc, bf_bc, "f")
            if cfg.use_lm_b:
                ones_t = tl_(sp, [1, 256], BF16, "ones")
                nc.vector.memset(ones_t[:], 1.0)
                lmb_sb = tl_(sp, [1, V], BF16, "lmb_sb")
                nc.sync.dma_start(lmb_sb[:], lmb_d[None, :])
            expacc = tl_(sp, [128, 2, VG * NVG], F32, "expacc")
            for vg in range(NVG):
                lm_t = []
                for k in range(DT):
                    t = tl_(wp, [128, VC * VG], BF16, "lm", "lm_t")
                    nc.sync.dma_start(
                        t[:], lm_d[k][:, vg * VC * VG:(vg + 1) * VC * VG])
                    lm_t.append(t)
                for c in range(VG):
                    col0 = vg * VG * VC + c * VC
                    for qt in range(2):
                        pl = pt_(ps, [128, VC], F32, "pbig")
                        for k in range(DT):
                            nc.tensor.matmul(
                                pl[:], hf_fm[:, k, qt * 128:(qt + 1) * 128],
                                lm_t[k][:, c * VC:(c + 1) * VC],
                                start=(k == 0),
                                stop=(k == DT - 1 and not cfg.use_lm_b))
                        if cfg.use_lm_b:
                            nc.tensor.matmul(
                                pl[:], ones_t[:, qt * 128:(qt + 1) * 128],
                                lmb_sb[:, col0:col0 + VC],
                                start=False, stop=True)
                        lsb = tl_(sp, [128, VC], F32, "lsb")
                        nc.vector.tensor_copy(lsb[:], pl[:])
                        nc.sync.dma_start(
                            logits_out[qt * 128:(qt + 1) * 128,
                                       col0:col0 + VC], lsb[:])
                        esc = tl_(sp, [128, VC], BF16, "esc")
                        nc.scalar.activation(
                            esc[:], pl[:], ACTF.Exp,
                            accum_out=expacc[:, qt,
                                             vg * VG + c:vg * VG + c + 1])
            # lse and loss terms
            lt_all = tl_(sp, [128, 2], F32, "lt")
            for qt in range(2):
                stot = tl_(sp, [128, 1], F32, "fstot")
                nc.vector.tensor_reduce(stot[:], expacc[:, qt, :], axis=AX.X,
                                        op=ALU.add)
                lse = tl_(sp, [128, 1], F32, "flse")
                nc.scalar.activation(lse[:], stot[:], ACTF.Ln)
                # gather target logits back from DRAM logits
                grow = tl_(sp, [128, 1, 512], F32, "grow")
                src = logits_out[qt * 128:(qt + 1) * 128, :].rearrange(
                    "a b -> (a b)").rearrange("(n e) -> n e", e=512)
                gidx = tl_(sp, [128, 8], I16, "gidx")
                nc.sync.dma_start(gidx[:], tgt_idx[:, qt * 8:(qt + 1) * 8])
                nc.gpsimd.dma_gather(grow[:], src, gidx[:], num_idxs=128,
                                     num_idxs_reg=128, elem_size=512)
                tm_sb = tl_(sp, [128, 512], F32, "tm_sb")
                nc.sync.dma_start(tm_sb[:], tmask[:, qt, :])
                prod = tl_(sp, [128, 512], F32, "prod")
                nc.vector.tensor_mul(prod[:], grow[:, 0, :], tm_sb[:])
                tlg = tl_(sp, [128, 1], F32, "tl")
                nc.vector.tensor_reduce(tlg[:], prod[:], axis=AX.X, op=ALU.add)
                nc.vector.tensor_sub(lt_all[:, qt:qt + 1], lse[:], tlg[:])
            nc.sync.dma_start(loss_out[:], lt_all[:])

    nc.compile()
    return nc


# ---------------------------------------------------------------------------
# host-side prep / run
# ---------------------------------------------------------------------------

def _prep_shared(cfg, tok_emb, Wq, Wk, Wv, Wo, W1, W2, lm_W, lm_b,
                 ln1_g, ln1_b, ln2_g, ln2_b, lnf_g, lnf_b, bo, b1, b2):
    D, L, FF, V, DT, FT = cfg.D, cfg.L, cfg.FF, cfg.V, cfg.DT, cfg.FT
    MG = FT // 4
    sh = {
        "tok_emb": np.ascontiguousarray(tok_emb, dtype=np.float32),
        "wq": np.ascontiguousarray(Wq.reshape(L, DT, 128, D).astype(BF)),
        "wk": np.ascontiguousarray(Wk.reshape(L, DT, 128, D).astype(BF)),
        "wv": np.ascontiguousarray(Wv.reshape(L, DT, 128, D).astype(BF)),
        "wo": np.ascontiguousarray(Wo.reshape(L, DT, 128, D).astype(BF)),
        "w1": np.ascontiguousarray(
            W1.reshape(L, DT, 128, MG, 512).transpose(0, 1, 3, 2, 4)
            .astype(BF)),
        "w2": np.ascontiguousarray(W2.reshape(L, FT, 128, D).astype(BF)),
        "lmw": np.ascontiguousarray(lm_W.reshape(DT, 128, V).astype(BF)),
    }
    if cfg.use_ln_affine:
        sh["lnp"] = np.ascontiguousarray(
            np.stack([ln1_g, ln1_b, ln2_g, ln2_b], axis=1), dtype=np.float32)
    if cfg.use_lnf_affine:
        sh["lnf"] = np.ascontiguousarray(
            np.stack([lnf_g, lnf_b]), dtype=np.float32)
    if cfg.use_b1:
        sh["b1f"] = np.ascontiguousarray(
            b1.reshape(L, FT, 128).transpose(2, 0, 1), dtype=np.float32)
    if cfg.use_bo:
        sh["bo_in"] = np.ascontiguousarray(bo, dtype=np.float32)
    if cfg.use_b2:
        sh["b2_in"] = np.ascontiguousarray(b2, dtype=np.float32)
    if cfg.use_lm_b:
        sh["lmb"] = np.ascontiguousarray(lm_b.astype(BF))
    return sh


def _wrap16(idx, width):
    """[n] -> [128, n//16]: 16-partition wrapped layout (idx i at
    [i%16, i//16]), replicated to 128 partitions."""
    n = idx.shape[0]
    out = np.zeros((16, width), dtype=np.int16)
    for i in range(n):
        out[i % 16, i // 16] = idx[i]
    return np.ascontiguousarray(np.tile(out, (8, 1)))


def _prep_core(cfg, c, x, targets, pos_emb):
    D, V = cfg.D, cfg.V
    b, r = c // 4, c % 4
    j0, j1 = r, 7 - r
    rows0 = slice(128 * j0, 128 * j0 + 128)
    rows1 = slice(128 * j1, 128 * j1 + 128)
    toks = np.concatenate([x[b, rows0], x[b, rows1]]).astype(np.int64)
    tgts = np.concatenate([targets[b, rows0], targets[b, rows1]]).astype(np.int64)
    pos = np.concatenate([pos_emb[rows0], pos_emb[rows1]])  # [256, D]

    per = {
        "tok_idx": _wrap16(toks.astype(np.int16), 16),
        "pos_sl": np.ascontiguousarray(
            pos.reshape(2, 128, D).transpose(1, 0, 2), dtype=np.float32),
    }
    # additive masks (0 = attend, -1e9 = masked); scores in AG block order
    p = np.arange(128)[:, None]
    xk = np.arange(128)[None, :]
    m0 = np.zeros((128, 4, 128), dtype=np.float32)
    for i in range(4):
        m0[:, i, :] = np.where(128 * i + xk <= 128 * j0 + p, 0.0, -1e9)
    m1 = np.zeros((128, 8, 128), dtype=np.float32)
    for bq in range(8):
        j = bq // 2 if bq % 2 == 0 else 7 - (bq - 1) // 2
        m1[:, bq, :] = np.where(128 * j + xk <= 128 * j1 + p, 0.0, -1e9)
    per["mask0"] = np.ascontiguousarray(m0.astype(BF))
    per["mask1"] = np.ascontiguousarray(m1.astype(BF))
    # target gather idx + mask
    tgt16 = np.zeros((128, 16), dtype=np.int16)
    tm = np.zeros((128, 2, 512), dtype=np.float32)
    for qt in range(2):
        tloc = tgts[qt * 128:(qt + 1) * 128]
        flat = np.arange(128) * V + tloc
        row512 = (flat // 512).astype(np.int16)
        within = flat % 512
        tgt16[:, qt * 8:(qt + 1) * 8] = _wrap16(row512, 8)
        tm[np.arange(128), qt, within] = 1.0
    per["tgt_idx"] = tgt16
    per["tmask"] = tm
    return per


_BUILD_CACHE = {}


def _get_nc(cfg):
    key = (cfg.D, cfg.L, cfg.FF, cfg.V, cfg.VC, cfg.VG, cfg.use_ln_affine,
           cfg.use_bo, cfg.use_b1, cfg.use_b2, cfg.use_lnf_affine,
           cfg.use_lm_b)
    if key not in _BUILD_CACHE:
        _BUILD_CACHE[key] = build(cfg)
    return _BUILD_CACHE[key]


def prepare(cfg, x, targets, tok_emb, pos_emb, Wq, Wk, Wv, Wo, bo, W1, b1,
            W2, b2, ln1_g, ln1_b, ln2_g, ln2_b, lnf_g, lnf_b, lm_W, lm_b):
    """Returns (nc, in_maps)."""
    x = np.asarray(x)
    targets = np.asarray(targets)
    args = [np.asarray(a, dtype=np.float32) for a in
            (tok_emb, pos_emb, Wq, Wk, Wv, Wo, bo, W1, b1, W2, b2,
             ln1_g, ln1_b, ln2_g, ln2_b, lnf_g, lnf_b, lm_W, lm_b)]
    (tok_emb, pos_emb, Wq, Wk, Wv, Wo, bo, W1, b1, W2, b2,
     ln1_g, ln1_b, ln2_g, ln2_b, lnf_g, lnf_b, lm_W, lm_b) = args

    cfg.use_ln_affine = not (np.all(ln1_g == 1) and np.all(ln1_b == 0)
                             and np.all(ln2_g == 1) and np.all(ln2_b == 0))
    cfg.use_lnf_affine = not (np.all(lnf_g == 1) and np.all(lnf_b == 0))
    cfg.use_bo = bool(np.any(bo))
    cfg.use_b1 = bool(np.any(b1))
    cfg.use_b2 = bool(np.any(b2))
    cfg.use_lm_b = bool(np.any(lm_b))

    nc = _get_nc(cfg)
    sh = _prep_shared(cfg, tok_emb, Wq, Wk, Wv, Wo, W1, W2, lm_W, lm_b,
                      ln1_g, ln1_b, ln2_g, ln2_b, lnf_g, lnf_b, bo, b1, b2)
    in_maps = []
    for c in range(N_CORES):
        m = dict(sh)
        m.update(_prep_core(cfg, c, x, targets, pos_emb))
        in_maps.append(m)
    return nc, in_maps


def assemble(cfg, results):
    B, T, V = cfg.B, cfg.T, cfg.V
    logits = np.zeros((B, T, V), dtype=np.float32)
    loss_sum = 0.0
    for c in range(N_CORES):
        b, r = c // 4, c % 4
        lo = results[c]["logits_out"]
        logits[b, 128 * r:128 * r + 128] = lo[:128]
        logits[b, 128 * (7 - r):128 * (7 - r) + 128] = lo[128:]
        loss_sum += float(np.asarray(results[c]["loss_terms"],
                                     dtype=np.float64).sum())
    loss = np.float32(loss_sum / (B * T))
    return logits, loss


def kernel(x, targets, tok_emb, pos_emb, Wq, Wk, Wv, Wo, bo, W1, b1, W2, b2,
           ln1_g, ln1_b, ln2_g, ln2_b, lnf_g, lnf_b, lm_W, lm_b):
    cfg = Cfg()
    nc, in_maps = prepare(cfg, x, targets, tok_emb, pos_emb, Wq, Wk, Wv, Wo,
                          bo, W1, b1, W2, b2, ln1_g, ln1_b, ln2_g, ln2_b,
                          lnf_g, lnf_b, lm_W, lm_b)
    res = bass_utils.run_bass_kernel_spmd(
        nc, in_maps, core_ids=list(range(N_CORES)), trace=False)
    return assemble(cfg, res.results)
